# revision 23
# baseline (speedup 1.0000x reference)
"""MobileAttention3D Trainium2 kernel (8-core SPMD), v2.

Sharding: core c -> (b = c//4, hg = c%4) owns batch b and H rows
[8*hg, 8*hg+8).  All conv GEMMs + attention for that slice run locally;
the only cross-core communication is a 32KB AllReduce of partial
attention logits within each batch group {0..3}, {4..7}.

v2 changes vs v1 (trace-driven):
  * Q2 free layout -> (dq, eta, kd, n): the q-conv PSUM drain becomes a
    single fully-contiguous 1024-el copy (was 5 ns/el scatter).
  * logits via swapped operands: lhsT = k chunks (32 cols), rhs = Q2
    stream (N=256), 4x column-tiled across PSUM partition strips; strip
    sums + transpose back to [nq, dk] via 4 concurrent fp32 PE
    row-tiles + DVE adds.  Kills 256 128-col LDWEIGHTS.
  * v-transpose done by SBUF->SBUF DMA (128B runs) instead of 128 PE
    transposes + scatter copies; lands directly in the 4-strip layout.
  * attn^T replicated to 4 partition strips with one col-tiled matmul
    quad; attention*V runs 4x row-tiled (contraction dk=32).
  * PSUM evacuation copies alternate between vector and scalar engines.
Output token order is (w', dq, n); host unshard adapts.
"""

import numpy as np
import ml_dtypes

NH, KD, VD, C = 8, 64, 64, 256
B, D, H, W = 2, 32, 32, 32
HS = H // 4            # h rows per core
T = D * HS * W         # 8192 tokens per core
P = 128
NCORES = 8
SCALE = float(VD) ** -0.5

_CACHE = {}


def _build(has_qb, has_kvb, has_pb, sim_mode=False):
    import concourse.bacc as bacc
    import concourse.mybir as mybir
    from concourse import tile

    dt = mybir.dt
    f32, bf16 = dt.float32, dt.bfloat16
    AX = mybir.AxisListType
    AF = mybir.ActivationFunctionType

    nc = bacc.Bacc("TRN2", target_bir_lowering=False, debug=False,
                   enable_asserts=False,
                   num_devices=1 if sim_mode else NCORES)

    x_in = nc.dram_tensor("x", [C, T], bf16, kind="ExternalInput")
    wq_in = nc.dram_tensor("wq", [C, NH * KD], bf16, kind="ExternalInput")
    wkv_in = nc.dram_tensor("wkv", [C, KD + VD], bf16, kind="ExternalInput")
    wp_in = nc.dram_tensor("wp", [NH * VD, C], bf16, kind="ExternalInput")
    idt_in = nc.dram_tensor("idt", [P, P], bf16, kind="ExternalInput")
    idtf_in = nc.dram_tensor("idtf", [P, 32], f32, kind="ExternalInput")
    qb_in = kvb_in = pb_in = None
    if has_qb:
        qb_in = nc.dram_tensor("qb", [P, NH * KD], bf16, kind="ExternalInput")
    if has_kvb:
        kvb_in = nc.dram_tensor("kvb", [P, KD + VD], bf16, kind="ExternalInput")
    if has_pb:
        # proj bias pre-multiplied by layer_scale, per C channel
        pb_in = nc.dram_tensor("pb", [C, 1], f32, kind="ExternalInput")
    out_t = nc.dram_tensor("out", [C, T], f32, kind="ExternalOutput")

    with tile.TileContext(nc) as tc:
        with tc.tile_pool(name="wpool", bufs=1) as wpool, \
             tc.tile_pool(name="big", bufs=1) as bigpool, \
             tc.tile_pool(name="q2p", bufs=1) as q2pool, \
             tc.tile_pool(name="kvp", bufs=1) as kvpool, \
             tc.tile_pool(name="small", bufs=1) as spool, \
             tc.tile_pool(name="stage", bufs=4) as stpool, \
             tc.tile_pool(name="psum", bufs=8, space="PSUM") as psum, \
             tc.tile_pool(name="dram", bufs=1, space="DRAM") as dram:

            # engine rotation for PSUM evacuation copies.
            # (GPSIMD cannot access PSUM, so only vector+scalar rotate.)
            rot_engines = [nc.vector, nc.scalar]
            rot_state = [0]

            def rot_copy(dst, src):
                eng = rot_engines[rot_state[0] % 2]
                rot_state[0] += 1
                if eng is nc.scalar:
                    eng.copy(dst, src)
                else:
                    eng.tensor_copy(dst, src)

            def rot_tt(dst, a, b_, op):
                nc.vector.tensor_tensor(dst, a, b_, op=op)

            # ---- load weights / constants ----
            wq = wpool.tile([P, 2, NH * KD], bf16)
            wkv = wpool.tile([P, 2, KD + VD], bf16)
            wp = wpool.tile([P, 4, C], bf16)
            idt = wpool.tile([P, P], bf16)
            idtf = wpool.tile([P, 32], f32)
            # wkv/wq are needed first; everything else queues behind x so
            # the x chunks (which pace the convs) aren't delayed
            for ci in range(2):
                nc.sync.dma_start(wkv[:, ci, :], wkv_in[ci * P:(ci + 1) * P, :])
                nc.sync.dma_start(wq[:, ci, :], wq_in[ci * P:(ci + 1) * P, :])
            qb = kvb = pb = None
            if has_qb:
                qb = wpool.tile([P, NH * KD], bf16)
                nc.sync.dma_start(qb[:], qb_in[:])
            if has_kvb:
                kvb = wpool.tile([P, KD + VD], bf16)
                nc.sync.dma_start(kvb[:], kvb_in[:])
            if has_pb:
                pb = wpool.tile([P, 2, 1], f32)
                for ci in range(2):
                    nc.sync.dma_start(pb[:, ci, :], pb_in[ci * P:(ci + 1) * P, :])

            # big slot shared sequentially: x (32KB/p) then oo (64KB/p)
            x_sb = bigpool.tile([P, 2, T], bf16, tag="big")
            XCH = 16
            for g in range(XCH):
                lo, hi = g * (T // XCH), (g + 1) * (T // XCH)
                for ci in range(2):
                    eng = nc.sync if ci == 0 else nc.scalar
                    eng.dma_start(x_sb[:, ci, lo:hi],
                                  x_in[ci * P:(ci + 1) * P, lo:hi])
            nc.scalar.dma_start(idtf[:], idtf_in[:])
            nc.scalar.dma_start(idt[:], idt_in[:])
            for jq in range(4):
                nc.scalar.dma_start(wp[:, jq, :], wp_in[jq * P:(jq + 1) * P, :])

            # Q2 [p=hw128, (dq, eta, kd, n)] -- contiguous 1024-el drains
            Q2 = q2pool.tile([P, 32 * 1024], bf16)
            ksb = kvpool.tile([P, 64 * KD], bf16)      # [p=hw128, (dk, eta, kd)]
            vsb = kvpool.tile([P, 64 * VD], bf16)      # [p=hw128, (dk, eta, vd)]
            # vatt4: strip r=[32r..32r+32) holds [dk, (eta, q, b, vd)] for
            # hw128 in [32r, 32r+32);  q = hw128%32 // 2, b = hw128%2
            vatt4 = kvpool.tile([P, 2 * 16 * 2 * VD], bf16)
            attn = spool.tile([P, 2, 32], bf16)
            attnT4 = spool.tile([P, 2, P], bf16)       # attn^T replicated 4 strips
            l2s = spool.tile([P, 256], f32)            # logits strips (dk, nq')
            lsum0 = spool.tile([P, 2, 32], f32)
            lsum1 = spool.tile([P, 2, 32], f32)
            l2 = spool.tile([P, 64], f32)
            l3 = spool.tile([P, 64], f32)
            ex = spool.tile([P, 2, 32], f32)
            red = spool.tile([P, 8], f32)

            arin = [dram.tile([P, 32], f32, name=f"arin{mu}")
                    for mu in range(2)]
            arout = [dram.tile([P, 32], f32, name=f"arout{mu}")
                     for mu in range(2)]

            # ---- kv conv (tokens on partitions), tracking x DMA arrival;
            # q conv for mu0's dq range rides along with its x chunk so
            # logits-mu0/AR0 can trigger right after the last kv chunk ----
            def q_conv(dq):
                for eta in range(2):
                    psq = psum.tile([P, 512], f32, tag="ps",
                                    name=f"psq{dq}_{eta}")
                    j = dq * 2 + eta
                    for ci in range(2):
                        nc.tensor.matmul(psq[:],
                                         x_sb[:, ci, j * P:(j + 1) * P],
                                         wq[:, ci, :],
                                         start=(ci == 0), stop=(ci == 1))
                    # psum free = (kd, n) [wq host col order]; dst is a
                    # contiguous 512-el slice of Q2 (dq, eta, kd, n)
                    dst = Q2[:, dq * 1024 + eta * 512:
                             dq * 1024 + (eta + 1) * 512]
                    if has_qb:
                        rot_tt(dst.rearrange("p c -> p 1 c"),
                               psq.rearrange("p c -> p 1 c"),
                               qb.rearrange("p c -> p 1 c"),
                               mybir.AluOpType.add)
                    else:
                        rot_copy(dst, psq[:])

            for m in range(16):
                ps = psum.tile([P, 512], f32, tag="ps", name=f"pskv{m}")
                for jj in range(4):
                    j = 4 * m + jj
                    for ci in range(2):
                        nc.tensor.matmul(
                            ps[:, jj * P:(jj + 1) * P],
                            x_sb[:, ci, j * P:(j + 1) * P],
                            wkv[:, ci, :],
                            start=(ci == 0), stop=(ci == 1))
                psv = ps.rearrange("p (t c) -> p t c", c=P)
                ks = ksb[:, m * 256:(m + 1) * 256].rearrange("p (t c) -> p t c", c=KD)
                vs = vsb[:, m * 256:(m + 1) * 256].rearrange("p (t c) -> p t c", c=VD)
                if has_kvb:
                    kvbv = kvb.rearrange("p c -> p 1 c")
                    rot_tt(ks, psv[:, :, 0:KD],
                           kvbv[:, [0, 0, 0, 0], 0:KD], mybir.AluOpType.add)
                    rot_tt(vs, psv[:, :, KD:KD + VD],
                           kvbv[:, [0, 0, 0, 0], KD:KD + VD], mybir.AluOpType.add)
                else:
                    rot_copy(ks, psv[:, :, 0:KD])
                    rot_copy(vs, psv[:, :, KD:KD + VD])
                if m < 8:
                    q_conv(2 * m)
                    q_conv(2 * m + 1)

            # ---- v "transpose" into vatt4 via DMA (DRAM bounce so every
            # SBUF AP is partition-first); overlaps the q conv ----
            vtd = [dram.tile([2, 32, 2048], bf16, name=f"vtd{r}")
                   for r in range(4)]
            for r in range(4):
                src1 = vsb[32 * r:32 * (r + 1), :].rearrange(
                    "qb (k e v) -> e qb k v", k=32, e=2, v=VD)
                dst1 = vtd[r].rearrange("e k (qb v) -> e qb k v",
                                        qb=32, v=VD)
                for eta in range(2):
                    # keep the whole bounce off the sync queue: sync carries
                    # arin/l3 + output stores on the critical path
                    nc.scalar.dma_start(dst1[eta], src1[eta])
            for r in range(4):
                for eta in range(2):
                    nc.scalar.dma_start(
                        vatt4[32 * r:32 * (r + 1),
                              eta * 2048:(eta + 1) * 2048],
                        vtd[r][eta])

            # ---- per-mu: q conv half -> logits half -> AllReduce ----
            # nq' = dq*8+n, so mu = dq-half: logits/AR for mu0 launch after
            # only half the q conv, hiding both AR latencies under compute.
            Q2v = Q2.rearrange("p (dq e k n) -> p e k dq n",
                               dq=32, e=2, k=KD, n=NH)
            ksv = ksb.rearrange("p (dk e k) -> p e k dk", e=2, k=KD)
            for mu in range(2):
                if mu == 1:
                    for dq in range(16, 32):
                        q_conv(dq)

                # logits, 4x col-tiled: lhsT = k chunk (32 cols), rhs = Q2
                # stream over this mu's 16 dq (N=128); strip c_ accumulates
                # (eta,kd) idx in [32c_, 32c_+32)
                psL2 = psum.tile([P, P], f32, tag="ps", name=f"psL2_{mu}")
                for step in range(32):
                    for c_ in range(4):
                        idx = c_ * 32 + step
                        eta, kd = idx // KD, idx % KD
                        nc.tensor.matmul(
                            psL2[32 * c_:32 * (c_ + 1), :],
                            ksv[:, eta, kd, :],
                            Q2v[:, eta, kd, mu * 16:(mu + 1) * 16, :],
                            start=(step == 0), stop=(step == 31),
                            tile_position=(0, 32 * c_), skip_group_check=True)
                nc.vector.tensor_copy(l2s[:, mu * P:(mu + 1) * P], psL2[:])

                # strip sums + transpose to [nq', dk] via 4 fp32 row-tiles
                lt = [psum.tile([P, 32], f32, tag="ps", name=f"lt{mu}_{c_}")
                      for c_ in range(4)]
                for c_ in range(4):
                    nc.tensor.matmul(
                        lt[c_][:],
                        l2s[32 * c_:32 * (c_ + 1), mu * P:(mu + 1) * P],
                        idtf[32 * c_:32 * (c_ + 1), :],
                        start=True, stop=True,
                        tile_position=(32 * c_, 0))
                # <=1 PSUM operand per DVE op: stage lt0/lt2 through SBUF
                nc.vector.tensor_copy(lsum0[:, mu, :], lt[0][:])
                nc.scalar.copy(lsum1[:, mu, :], lt[2][:])
                nc.vector.tensor_tensor(lsum0[:, mu, :], lsum0[:, mu, :],
                                        lt[1][:], op=mybir.AluOpType.add)
                nc.vector.tensor_tensor(lsum1[:, mu, :], lsum1[:, mu, :],
                                        lt[3][:], op=mybir.AluOpType.add)
                nc.vector.tensor_tensor(l2[:, mu * 32:(mu + 1) * 32],
                                        lsum0[:, mu, :], lsum1[:, mu, :],
                                        op=mybir.AluOpType.add)
                nc.sync.dma_start(arin[mu][:], l2[:, mu * 32:(mu + 1) * 32])
                if sim_mode:
                    nc.sync.dma_start(arout[mu][:], arin[mu][:])
                else:
                    nc.gpsimd.collective_compute(
                        "AllReduce", mybir.AluOpType.add,
                        replica_groups=[[0, 1, 2, 3], [4, 5, 6, 7]],
                        ins=[arin[mu].opt()], outs=[arout[mu].opt()])
                nc.sync.dma_start(l3[:, mu * 32:(mu + 1) * 32], arout[mu][:])

            oo = bigpool.tile([P, 4, T], bf16, tag="big", name="oo")
            # oo free per jq plane: f' = w'*256 + nq',  nq' = dq*8 + n
            oov = oo.rearrange("p jq (wh wl n) -> p jq wl wh n", wh=8, wl=4)

            def av_group(mu, eta, qh):
                # tile r holds strip r's outputs for all 4 jq (one PSUM
                # bank per row-tile; concurrent row-tiles never share one)
                pr = [psum.tile([P, 512], f32, tag="ps",
                                name=f"psav{mu}_{eta}_{qh}_{r}")
                      for r in range(4)]
                for jq in range(4):
                    q_ = qh * 4 + jq
                    for r in range(4):
                        nc.tensor.matmul(
                            pr[r][:, jq * P:(jq + 1) * P],
                            vatt4[32 * r:32 * (r + 1),
                                  eta * 2048 + q_ * P:eta * 2048 + (q_ + 1) * P],
                            attnT4[32 * r:32 * (r + 1), mu, :],
                            start=True, stop=True,
                            tile_position=(32 * r, 0))
                for r in range(4):
                    # [p, (jq, nq)] -> oo planes jq at w' = eta*16+4r+qh
                    rot_copy(
                        oov[:, :, qh, eta * 4 + r, mu * P:(mu + 1) * P],
                        pr[r].rearrange("p (jq n) -> p jq n", jq=4))

            # out token order t = mu*4096 + w'*128 + dqloc*8 + n, so the
            # whole mu0 proj + store pipeline runs under the AR1 window
            outv = out_t.rearrange("(ct p) t -> p ct t", p=P)
            oow = [oo[:, jq, :].rearrange("p (w q) -> p w q", w=32)
                   for jq in range(4)]

            def proj_quad(mu, tq):
                # output tokens w' in [4tq, 4tq+4), nq'-half mu
                stg = stpool.tile([P, 2, 512], f32, tag="stg",
                                  name=f"stg{mu}_{tq}")
                for ct in range(2):
                    ps = psum.tile([P, 512], f32, tag="ps",
                                   name=f"psp{mu}_{tq}_{ct}")
                    for jq in range(4):
                        nc.tensor.matmul(
                            ps[:],
                            wp[:, jq, ct * P:(ct + 1) * P],
                            oow[jq][:, 4 * tq:4 * (tq + 1),
                                    mu * P:(mu + 1) * P],
                            start=(jq == 0), stop=(jq == 3))
                    if has_pb:
                        eng = rot_engines[rot_state[0] % 2]
                        rot_state[0] += 1
                        eng.tensor_scalar_add(stg[:, ct, :], ps[:],
                                              pb[:, ct, :])
                    else:
                        rot_copy(stg[:, ct, :], ps[:])
                base = mu * 4096 + tq * 512
                if mu == 1 and tq == 7:
                    for ct in range(2):
                        eng = nc.sync if ct == 0 else nc.scalar
                        eng.dma_start(outv[:, ct, base:base + 512],
                                      stg[:, ct, :])
                else:
                    eng = nc.sync if tq % 2 == 0 else nc.scalar
                    eng.dma_start(outv[:, :, base:base + 512], stg[:])

            for mu in range(2):
                # ---- softmax over dk (free axis) ----
                sl = l3[:, mu * 32:(mu + 1) * 32]
                mx = red[:, mu * 4 + 0: mu * 4 + 1]
                mxn = red[:, mu * 4 + 1: mu * 4 + 2]
                sm = red[:, mu * 4 + 2: mu * 4 + 3]
                rs = red[:, mu * 4 + 3: mu * 4 + 4]
                nc.vector.reduce_max(mx, sl, axis=AX.X, op=mybir.AluOpType.max)
                nc.scalar.mul(mxn, mx, -SCALE)
                nc.scalar.activation(ex[:, mu, :], sl, AF.Exp,
                                     bias=mxn, scale=SCALE, accum_out=sm)
                nc.vector.reciprocal(rs, sm)
                nc.vector.tensor_scalar_mul(attn[:, mu, :], ex[:, mu, :], rs)

                # ---- attn^T replicated to 4 strips (col-tiled quad) ----
                psT = psum.tile([P, P], f32, tag="ps", name=f"psat{mu}")
                for r in range(4):
                    nc.tensor.matmul(psT[32 * r:32 * (r + 1), :],
                                     attn[:, mu, :], idt[:, 0:P],
                                     start=True, stop=True,
                                     tile_position=(0, 32 * r))
                nc.vector.tensor_copy(attnT4[:, mu, :], psT[:])

                # ---- attention * V, then proj per w'-quad ----
                # proj quad tq reads av strip r = tq%4 of eta = tq//4 for
                # all qh, so run all 4 qh groups of an eta, then its projs
                for eta in range(2):
                    for qh in range(4):
                        av_group(mu, eta, qh)
                    for tq in range(eta * 4, (eta + 1) * 4):
                        proj_quad(mu, tq)

    nc.finalize()
    return nc


def _get_nc(has_qb, has_kvb, has_pb, sim_mode=False):
    key = (has_qb, has_kvb, has_pb, sim_mode)
    if key not in _CACHE:
        _CACHE[key] = _build(*key)
    return _CACHE[key]


def _host_inputs(q_w, q_b, kv_w, kv_b, proj_w, proj_b, layer_scale,
                 has_qb, has_kvb, has_pb):
    bf = ml_dtypes.bfloat16
    ls_c = layer_scale.reshape(C)                          # [C] f32
    # wq columns reordered to (kd, n) so the q-conv drain is contiguous
    wq = np.ascontiguousarray(
        q_w.reshape(NH, KD, C).transpose(2, 1, 0).reshape(C, NH * KD)
    ).astype(bf)
    wkv = np.ascontiguousarray(kv_w.T).astype(bf)          # [C, 128]
    wp = np.ascontiguousarray((proj_w * ls_c[:, None]).T).astype(bf)
    idt = np.eye(P, dtype=bf)
    idtf = np.tile(np.eye(32, dtype=np.float32), (4, 1))   # [128, 32]

    shared = {"wq": wq, "wkv": wkv, "wp": wp, "idt": idt, "idtf": idtf}
    if has_qb:
        qbr = q_b.reshape(NH, KD).T.reshape(NH * KD)
        shared["qb"] = np.broadcast_to(qbr.astype(bf), (P, NH * KD)).copy()
    if has_kvb:
        shared["kvb"] = np.broadcast_to(kv_b.astype(bf), (P, KD + VD)).copy()
    if has_pb:
        shared["pb"] = (proj_b * layer_scale.reshape(-1)).reshape(C, 1) \
            .astype(np.float32)
    return shared


def kernel(x, q_w, q_b, kv_w, kv_b, proj_w, proj_b, layer_scale):
    from concourse.bass_utils import run_bass_kernel_spmd
    import os

    x = np.asarray(x, dtype=np.float32)
    q_w = np.asarray(q_w, dtype=np.float32)
    q_b = np.asarray(q_b, dtype=np.float32)
    kv_w = np.asarray(kv_w, dtype=np.float32)
    kv_b = np.asarray(kv_b, dtype=np.float32)
    proj_w = np.asarray(proj_w, dtype=np.float32)
    proj_b = np.asarray(proj_b, dtype=np.float32)
    layer_scale = np.asarray(layer_scale, dtype=np.float32)

    has_qb = bool(np.any(q_b != 0))
    has_kvb = bool(np.any(kv_b != 0))
    has_pb = bool(np.any(proj_b != 0))
    nc = _get_nc(has_qb, has_kvb, has_pb)

    bf = ml_dtypes.bfloat16
    shared = _host_inputs(q_w, q_b, kv_w, kv_b, proj_w, proj_b, layer_scale,
                          has_qb, has_kvb, has_pb)

    in_maps = []
    for c in range(NCORES):
        b, hg = c // 4, c % 4
        xc = np.ascontiguousarray(
            x[b, :, :, hg * HS:(hg + 1) * HS, :].reshape(C, T)).astype(bf)
        in_maps.append({"x": xc, **shared})

    trace = bool(int(os.environ.get("KERNEL_TRACE", "0")))
    res = run_bass_kernel_spmd(nc, in_maps, core_ids=list(range(NCORES)),
                               trace=trace)
    kernel.last_results = res

    out = np.empty((B, C, D, H, W), dtype=np.float32)
    for c in range(NCORES):
        b, hg = c // 4, c % 4
        # out token order: t = mu*4096 + w'*128 + dqloc*8 + n
        r = res.results[c]["out"].reshape(C, 2, W, 16, NH)
        for mu in range(2):
            out[b, :, mu * 16:(mu + 1) * 16, hg::4, :] = \
                r[:, mu].transpose(0, 2, 3, 1)
    return out


# revision 24
# speedup vs baseline: 1.1848x; 1.1848x over previous
"""MobileAttention3D Trainium2 kernel (8-core SPMD), v2.

Sharding: core c -> (b = c//4, hg = c%4) owns batch b and H rows
[8*hg, 8*hg+8).  All conv GEMMs + attention for that slice run locally;
the only cross-core communication is a 32KB AllReduce of partial
attention logits within each batch group {0..3}, {4..7}.

v2 changes vs v1 (trace-driven):
  * Q2 free layout -> (dq, eta, kd, n): the q-conv PSUM drain becomes a
    single fully-contiguous 1024-el copy (was 5 ns/el scatter).
  * logits via swapped operands: lhsT = k chunks (32 cols), rhs = Q2
    stream (N=256), 4x column-tiled across PSUM partition strips; strip
    sums + transpose back to [nq, dk] via 4 concurrent fp32 PE
    row-tiles + DVE adds.  Kills 256 128-col LDWEIGHTS.
  * v-transpose done by SBUF->SBUF DMA (128B runs) instead of 128 PE
    transposes + scatter copies; lands directly in the 4-strip layout.
  * attn^T replicated to 4 partition strips with one col-tiled matmul
    quad; attention*V runs 4x row-tiled (contraction dk=32).
  * PSUM evacuation copies alternate between vector and scalar engines.
Output token order is (w', dq, n); host unshard adapts.
"""

import numpy as np
import ml_dtypes

NH, KD, VD, C = 8, 64, 64, 256
B, D, H, W = 2, 32, 32, 32
HS = H // 4            # h rows per core
T = D * HS * W         # 8192 tokens per core
P = 128
NCORES = 8
SCALE = float(VD) ** -0.5

_CACHE = {}


def _build(has_qb, has_kvb, has_pb, sim_mode=False):
    import concourse.bacc as bacc
    import concourse.mybir as mybir
    from concourse import tile

    dt = mybir.dt
    f32, bf16 = dt.float32, dt.bfloat16
    AX = mybir.AxisListType
    AF = mybir.ActivationFunctionType

    nc = bacc.Bacc("TRN2", target_bir_lowering=False, debug=False,
                   enable_asserts=False,
                   num_devices=1 if sim_mode else NCORES)

    x_in = nc.dram_tensor("x", [C, T], bf16, kind="ExternalInput")
    wq_in = nc.dram_tensor("wq", [C, NH * KD], bf16, kind="ExternalInput")
    wkv_in = nc.dram_tensor("wkv", [C, KD + VD], bf16, kind="ExternalInput")
    wp_in = nc.dram_tensor("wp", [NH * VD, C], bf16, kind="ExternalInput")
    idt_in = nc.dram_tensor("idt", [P, P], bf16, kind="ExternalInput")
    idtf_in = nc.dram_tensor("idtf", [P, 32], f32, kind="ExternalInput")
    qb_in = kvb_in = pb_in = None
    if has_qb:
        qb_in = nc.dram_tensor("qb", [P, NH * KD], bf16, kind="ExternalInput")
    if has_kvb:
        kvb_in = nc.dram_tensor("kvb", [P, KD + VD], bf16, kind="ExternalInput")
    if has_pb:
        # proj bias pre-multiplied by layer_scale, per C channel
        pb_in = nc.dram_tensor("pb", [C, 1], f32, kind="ExternalInput")
    out_t = nc.dram_tensor("out", [C, T], f32, kind="ExternalOutput")

    with tile.TileContext(nc) as tc:
        with tc.tile_pool(name="wpool", bufs=1) as wpool, \
             tc.tile_pool(name="big", bufs=1) as bigpool, \
             tc.tile_pool(name="q2p", bufs=1) as q2pool, \
             tc.tile_pool(name="kvp", bufs=1) as kvpool, \
             tc.tile_pool(name="small", bufs=1) as spool, \
             tc.tile_pool(name="stage", bufs=4) as stpool, \
             tc.tile_pool(name="psum", bufs=8, space="PSUM") as psum, \
             tc.tile_pool(name="dram", bufs=1, space="DRAM") as dram:

            # engine rotation for PSUM evacuation copies.
            # (GPSIMD cannot access PSUM, so only vector+scalar rotate.)
            rot_engines = [nc.vector, nc.scalar]
            rot_state = [0]

            def rot_copy(dst, src):
                eng = rot_engines[rot_state[0] % 2]
                rot_state[0] += 1
                if eng is nc.scalar:
                    eng.copy(dst, src)
                else:
                    eng.tensor_copy(dst, src)

            def rot_tt(dst, a, b_, op):
                nc.vector.tensor_tensor(dst, a, b_, op=op)

            # ---- load weights / constants ----
            wq = wpool.tile([P, 2, NH * KD], bf16)
            wkv = wpool.tile([P, 2, KD + VD], bf16)
            wp = wpool.tile([P, 4, C], bf16)
            idt = wpool.tile([P, P], bf16)
            idtf = wpool.tile([P, 32], f32)
            # wkv/wq are needed first; everything else queues behind x so
            # the x chunks (which pace the convs) aren't delayed
            for ci in range(2):
                nc.sync.dma_start(wkv[:, ci, :], wkv_in[ci * P:(ci + 1) * P, :])
                nc.sync.dma_start(wq[:, ci, :], wq_in[ci * P:(ci + 1) * P, :])
            qb = kvb = pb = None
            if has_qb:
                qb = wpool.tile([P, NH * KD], bf16)
                nc.sync.dma_start(qb[:], qb_in[:])
            if has_kvb:
                kvb = wpool.tile([P, KD + VD], bf16)
                nc.sync.dma_start(kvb[:], kvb_in[:])
            if has_pb:
                pb = wpool.tile([P, 2, 1], f32)
                for ci in range(2):
                    nc.sync.dma_start(pb[:, ci, :], pb_in[ci * P:(ci + 1) * P, :])

            # big slot shared sequentially: x (32KB/p) then oo (64KB/p)
            x_sb = bigpool.tile([P, 2, T], bf16, tag="big")
            XCH = 16
            for g in range(XCH):
                lo, hi = g * (T // XCH), (g + 1) * (T // XCH)
                for ci in range(2):
                    eng = nc.sync if ci == 0 else nc.scalar
                    eng.dma_start(x_sb[:, ci, lo:hi],
                                  x_in[ci * P:(ci + 1) * P, lo:hi])
            nc.sync.dma_start(idt[:], idt_in[:])
            nc.sync.dma_start(idtf[:], idtf_in[:])
            for jq in range(4):
                nc.sync.dma_start(wp[:, jq, :], wp_in[jq * P:(jq + 1) * P, :])

            # Q2 [p=hw128, (dq, eta, kd, n)] -- contiguous 1024-el drains
            Q2 = q2pool.tile([P, 32 * 1024], bf16)
            ksb = kvpool.tile([P, 64 * KD], bf16)      # [p=hw128, (dk, eta, kd)]
            vsb = kvpool.tile([P, 64 * VD], bf16)      # [p=hw128, (dk, eta, vd)]
            # vatt4: strip r=[32r..32r+32) holds [dk, (eta, q, b, vd)] for
            # hw128 in [32r, 32r+32);  q = hw128%32 // 2, b = hw128%2
            vatt4 = kvpool.tile([P, 2 * 16 * 2 * VD], bf16)
            attn = spool.tile([P, 2, 32], bf16)
            attnT4 = spool.tile([P, 2, P], bf16)       # attn^T replicated 4 strips
            l2s = spool.tile([P, 256], f32)            # logits strips (dk, nq')
            lsum0 = spool.tile([P, 2, 32], f32)
            lsum1 = spool.tile([P, 2, 32], f32)
            l2 = spool.tile([P, 64], f32)
            l3 = spool.tile([P, 64], f32)
            ex = spool.tile([P, 2, 32], f32)
            red = spool.tile([P, 8], f32)

            arin = [dram.tile([P, 32], f32, name=f"arin{mu}")
                    for mu in range(2)]
            arout = [dram.tile([P, 32], f32, name=f"arout{mu}")
                     for mu in range(2)]

            # ---- kv conv (tokens on partitions), tracking x DMA arrival;
            # q conv for mu0's dq range rides along with its x chunk so
            # logits-mu0/AR0 can trigger right after the last kv chunk ----
            def q_conv(dq):
                for eta in range(2):
                    psq = psum.tile([P, 512], f32, tag="ps",
                                    name=f"psq{dq}_{eta}")
                    j = dq * 2 + eta
                    for ci in range(2):
                        nc.tensor.matmul(psq[:],
                                         x_sb[:, ci, j * P:(j + 1) * P],
                                         wq[:, ci, :],
                                         start=(ci == 0), stop=(ci == 1))
                    # psum free = (kd, n) [wq host col order]; dst is a
                    # contiguous 512-el slice of Q2 (dq, eta, kd, n)
                    dst = Q2[:, dq * 1024 + eta * 512:
                             dq * 1024 + (eta + 1) * 512]
                    if has_qb:
                        rot_tt(dst.rearrange("p c -> p 1 c"),
                               psq.rearrange("p c -> p 1 c"),
                               qb.rearrange("p c -> p 1 c"),
                               mybir.AluOpType.add)
                    else:
                        rot_copy(dst, psq[:])

            for m in range(16):
                ps = psum.tile([P, 512], f32, tag="ps", name=f"pskv{m}")
                for jj in range(4):
                    j = 4 * m + jj
                    for ci in range(2):
                        nc.tensor.matmul(
                            ps[:, jj * P:(jj + 1) * P],
                            x_sb[:, ci, j * P:(j + 1) * P],
                            wkv[:, ci, :],
                            start=(ci == 0), stop=(ci == 1))
                psv = ps.rearrange("p (t c) -> p t c", c=P)
                ks = ksb[:, m * 256:(m + 1) * 256].rearrange("p (t c) -> p t c", c=KD)
                vs = vsb[:, m * 256:(m + 1) * 256].rearrange("p (t c) -> p t c", c=VD)
                if has_kvb:
                    kvbv = kvb.rearrange("p c -> p 1 c")
                    rot_tt(ks, psv[:, :, 0:KD],
                           kvbv[:, [0, 0, 0, 0], 0:KD], mybir.AluOpType.add)
                    rot_tt(vs, psv[:, :, KD:KD + VD],
                           kvbv[:, [0, 0, 0, 0], KD:KD + VD], mybir.AluOpType.add)
                else:
                    rot_copy(ks, psv[:, :, 0:KD])
                    rot_copy(vs, psv[:, :, KD:KD + VD])
                if m < 8:
                    q_conv(2 * m)
                    q_conv(2 * m + 1)

            # ---- v "transpose" into vatt4 via DMA (DRAM bounce so every
            # SBUF AP is partition-first); overlaps the q conv ----
            vtd = [dram.tile([2, 32, 2048], bf16, name=f"vtd{r}")
                   for r in range(4)]
            for r in range(4):
                src1 = vsb[32 * r:32 * (r + 1), :].rearrange(
                    "qb (k e v) -> e qb k v", k=32, e=2, v=VD)
                dst1 = vtd[r].rearrange("e k (qb v) -> e qb k v",
                                        qb=32, v=VD)
                for eta in range(2):
                    eng = nc.sync if (r + eta) % 2 == 0 else nc.scalar
                    eng.dma_start(dst1[eta], src1[eta])
            for r in range(4):
                for eta in range(2):
                    eng = nc.sync if (r + eta) % 2 == 1 else nc.scalar
                    eng.dma_start(
                        vatt4[32 * r:32 * (r + 1),
                              eta * 2048:(eta + 1) * 2048],
                        vtd[r][eta])

            # ---- per-mu: q conv half -> logits half -> AllReduce ----
            # nq' = dq*8+n, so mu = dq-half: logits/AR for mu0 launch after
            # only half the q conv, hiding both AR latencies under compute.
            Q2v = Q2.rearrange("p (dq e k n) -> p e k dq n",
                               dq=32, e=2, k=KD, n=NH)
            ksv = ksb.rearrange("p (dk e k) -> p e k dk", e=2, k=KD)
            for mu in range(2):
                if mu == 1:
                    for dq in range(16, 32):
                        q_conv(dq)

                # logits, 4x col-tiled: lhsT = k chunk (32 cols), rhs = Q2
                # stream over this mu's 16 dq (N=128); strip c_ accumulates
                # (eta,kd) idx in [32c_, 32c_+32)
                psL2 = psum.tile([P, P], f32, tag="ps", name=f"psL2_{mu}")
                for step in range(32):
                    for c_ in range(4):
                        idx = c_ * 32 + step
                        eta, kd = idx // KD, idx % KD
                        nc.tensor.matmul(
                            psL2[32 * c_:32 * (c_ + 1), :],
                            ksv[:, eta, kd, :],
                            Q2v[:, eta, kd, mu * 16:(mu + 1) * 16, :],
                            start=(step == 0), stop=(step == 31),
                            tile_position=(0, 32 * c_), skip_group_check=True)
                nc.vector.tensor_copy(l2s[:, mu * P:(mu + 1) * P], psL2[:])

                # strip sums + transpose to [nq', dk] via 4 fp32 row-tiles
                lt = [psum.tile([P, 32], f32, tag="ps", name=f"lt{mu}_{c_}")
                      for c_ in range(4)]
                for c_ in range(4):
                    nc.tensor.matmul(
                        lt[c_][:],
                        l2s[32 * c_:32 * (c_ + 1), mu * P:(mu + 1) * P],
                        idtf[32 * c_:32 * (c_ + 1), :],
                        start=True, stop=True,
                        tile_position=(32 * c_, 0))
                # <=1 PSUM operand per DVE op: stage lt0/lt2 through SBUF
                nc.vector.tensor_copy(lsum0[:, mu, :], lt[0][:])
                nc.scalar.copy(lsum1[:, mu, :], lt[2][:])
                nc.vector.tensor_tensor(lsum0[:, mu, :], lsum0[:, mu, :],
                                        lt[1][:], op=mybir.AluOpType.add)
                nc.vector.tensor_tensor(lsum1[:, mu, :], lsum1[:, mu, :],
                                        lt[3][:], op=mybir.AluOpType.add)
                nc.vector.tensor_tensor(l2[:, mu * 32:(mu + 1) * 32],
                                        lsum0[:, mu, :], lsum1[:, mu, :],
                                        op=mybir.AluOpType.add)
                nc.sync.dma_start(arin[mu][:], l2[:, mu * 32:(mu + 1) * 32])
                if sim_mode:
                    nc.sync.dma_start(arout[mu][:], arin[mu][:])
                else:
                    nc.gpsimd.collective_compute(
                        "AllReduce", mybir.AluOpType.add,
                        replica_groups=[[0, 1, 2, 3], [4, 5, 6, 7]],
                        ins=[arin[mu].opt()], outs=[arout[mu].opt()])
                nc.sync.dma_start(l3[:, mu * 32:(mu + 1) * 32], arout[mu][:])

            oo = bigpool.tile([P, 4, T], bf16, tag="big", name="oo")
            # oo free per jq plane: f' = w'*256 + nq',  nq' = dq*8 + n
            oov = oo.rearrange("p jq (wh wl n) -> p jq wl wh n", wh=8, wl=4)

            def av_group(mu, eta, qh):
                # tile r holds strip r's outputs for all 4 jq (one PSUM
                # bank per row-tile; concurrent row-tiles never share one)
                pr = [psum.tile([P, 512], f32, tag="ps",
                                name=f"psav{mu}_{eta}_{qh}_{r}")
                      for r in range(4)]
                for jq in range(4):
                    q_ = qh * 4 + jq
                    for r in range(4):
                        nc.tensor.matmul(
                            pr[r][:, jq * P:(jq + 1) * P],
                            vatt4[32 * r:32 * (r + 1),
                                  eta * 2048 + q_ * P:eta * 2048 + (q_ + 1) * P],
                            attnT4[32 * r:32 * (r + 1), mu, :],
                            start=True, stop=True,
                            tile_position=(32 * r, 0))
                for r in range(4):
                    # [p, (jq, nq)] -> oo planes jq at w' = eta*16+4r+qh
                    rot_copy(
                        oov[:, :, qh, eta * 4 + r, mu * P:(mu + 1) * P],
                        pr[r].rearrange("p (jq n) -> p jq n", jq=4))

            # out token order t = mu*4096 + w'*128 + dqloc*8 + n, so the
            # whole mu0 proj + store pipeline runs under the AR1 window
            outv = out_t.rearrange("(ct p) t -> p ct t", p=P)
            oow = [oo[:, jq, :].rearrange("p (w q) -> p w q", w=32)
                   for jq in range(4)]

            def proj_quad(mu, tq):
                # output tokens w' in [4tq, 4tq+4), nq'-half mu
                stg = stpool.tile([P, 2, 512], f32, tag="stg",
                                  name=f"stg{mu}_{tq}")
                for ct in range(2):
                    ps = psum.tile([P, 512], f32, tag="ps",
                                   name=f"psp{mu}_{tq}_{ct}")
                    for jq in range(4):
                        nc.tensor.matmul(
                            ps[:],
                            wp[:, jq, ct * P:(ct + 1) * P],
                            oow[jq][:, 4 * tq:4 * (tq + 1),
                                    mu * P:(mu + 1) * P],
                            start=(jq == 0), stop=(jq == 3))
                    if has_pb:
                        eng = rot_engines[rot_state[0] % 2]
                        rot_state[0] += 1
                        eng.tensor_scalar_add(stg[:, ct, :], ps[:],
                                              pb[:, ct, :])
                    else:
                        rot_copy(stg[:, ct, :], ps[:])
                base = mu * 4096 + tq * 512
                if mu == 1 and tq == 7:
                    for ct in range(2):
                        eng = nc.sync if ct == 0 else nc.scalar
                        eng.dma_start(outv[:, ct, base:base + 512],
                                      stg[:, ct, :])
                else:
                    eng = nc.sync if tq % 2 == 0 else nc.scalar
                    eng.dma_start(outv[:, :, base:base + 512], stg[:])

            for mu in range(2):
                # ---- softmax over dk (free axis) ----
                sl = l3[:, mu * 32:(mu + 1) * 32]
                mx = red[:, mu * 4 + 0: mu * 4 + 1]
                mxn = red[:, mu * 4 + 1: mu * 4 + 2]
                sm = red[:, mu * 4 + 2: mu * 4 + 3]
                rs = red[:, mu * 4 + 3: mu * 4 + 4]
                nc.vector.reduce_max(mx, sl, axis=AX.X, op=mybir.AluOpType.max)
                nc.scalar.mul(mxn, mx, -SCALE)
                nc.scalar.activation(ex[:, mu, :], sl, AF.Exp,
                                     bias=mxn, scale=SCALE, accum_out=sm)
                nc.vector.reciprocal(rs, sm)
                nc.vector.tensor_scalar_mul(attn[:, mu, :], ex[:, mu, :], rs)

                # ---- attn^T replicated to 4 strips (col-tiled quad) ----
                psT = psum.tile([P, P], f32, tag="ps", name=f"psat{mu}")
                for r in range(4):
                    nc.tensor.matmul(psT[32 * r:32 * (r + 1), :],
                                     attn[:, mu, :], idt[:, 0:P],
                                     start=True, stop=True,
                                     tile_position=(0, 32 * r))
                nc.vector.tensor_copy(attnT4[:, mu, :], psT[:])

                # ---- attention * V, then proj per w'-quad ----
                # proj quad tq reads av strip r = tq%4 of eta = tq//4 for
                # all qh, so run all 4 qh groups of an eta, then its projs
                for eta in range(2):
                    for qh in range(4):
                        av_group(mu, eta, qh)
                    for tq in range(eta * 4, (eta + 1) * 4):
                        proj_quad(mu, tq)

    nc.finalize()
    return nc


def _get_nc(has_qb, has_kvb, has_pb, sim_mode=False):
    key = (has_qb, has_kvb, has_pb, sim_mode)
    if key not in _CACHE:
        _CACHE[key] = _build(*key)
    return _CACHE[key]


def _host_inputs(q_w, q_b, kv_w, kv_b, proj_w, proj_b, layer_scale,
                 has_qb, has_kvb, has_pb):
    bf = ml_dtypes.bfloat16
    ls_c = layer_scale.reshape(C)                          # [C] f32
    # wq columns reordered to (kd, n) so the q-conv drain is contiguous
    wq = np.ascontiguousarray(
        q_w.reshape(NH, KD, C).transpose(2, 1, 0).reshape(C, NH * KD)
    ).astype(bf)
    wkv = np.ascontiguousarray(kv_w.T).astype(bf)          # [C, 128]
    wp = np.ascontiguousarray((proj_w * ls_c[:, None]).T).astype(bf)
    idt = np.eye(P, dtype=bf)
    idtf = np.tile(np.eye(32, dtype=np.float32), (4, 1))   # [128, 32]

    shared = {"wq": wq, "wkv": wkv, "wp": wp, "idt": idt, "idtf": idtf}
    if has_qb:
        qbr = q_b.reshape(NH, KD).T.reshape(NH * KD)
        shared["qb"] = np.broadcast_to(qbr.astype(bf), (P, NH * KD)).copy()
    if has_kvb:
        shared["kvb"] = np.broadcast_to(kv_b.astype(bf), (P, KD + VD)).copy()
    if has_pb:
        shared["pb"] = (proj_b * layer_scale.reshape(-1)).reshape(C, 1) \
            .astype(np.float32)
    return shared


def kernel(x, q_w, q_b, kv_w, kv_b, proj_w, proj_b, layer_scale):
    from concourse.bass_utils import run_bass_kernel_spmd
    import os

    x = np.asarray(x, dtype=np.float32)
    q_w = np.asarray(q_w, dtype=np.float32)
    q_b = np.asarray(q_b, dtype=np.float32)
    kv_w = np.asarray(kv_w, dtype=np.float32)
    kv_b = np.asarray(kv_b, dtype=np.float32)
    proj_w = np.asarray(proj_w, dtype=np.float32)
    proj_b = np.asarray(proj_b, dtype=np.float32)
    layer_scale = np.asarray(layer_scale, dtype=np.float32)

    has_qb = bool(np.any(q_b != 0))
    has_kvb = bool(np.any(kv_b != 0))
    has_pb = bool(np.any(proj_b != 0))
    nc = _get_nc(has_qb, has_kvb, has_pb)

    bf = ml_dtypes.bfloat16
    shared = _host_inputs(q_w, q_b, kv_w, kv_b, proj_w, proj_b, layer_scale,
                          has_qb, has_kvb, has_pb)

    in_maps = []
    for c in range(NCORES):
        b, hg = c // 4, c % 4
        xc = np.ascontiguousarray(
            x[b, :, :, hg * HS:(hg + 1) * HS, :].reshape(C, T)).astype(bf)
        in_maps.append({"x": xc, **shared})

    trace = bool(int(os.environ.get("KERNEL_TRACE", "0")))
    res = run_bass_kernel_spmd(nc, in_maps, core_ids=list(range(NCORES)),
                               trace=trace)
    kernel.last_results = res

    out = np.empty((B, C, D, H, W), dtype=np.float32)
    for c in range(NCORES):
        b, hg = c // 4, c % 4
        # out token order: t = mu*4096 + w'*128 + dqloc*8 + n
        r = res.results[c]["out"].reshape(C, 2, W, 16, NH)
        for mu in range(2):
            out[b, :, mu * 16:(mu + 1) * 16, hg::4, :] = \
                r[:, mu].transpose(0, 2, 3, 1)
    return out


# revision 25
# speedup vs baseline: 1.2192x; 1.0291x over previous
"""MobileAttention3D Trainium2 kernel (8-core SPMD), v2.

Sharding: core c -> (b = c//4, hg = c%4) owns batch b and H rows
[8*hg, 8*hg+8).  All conv GEMMs + attention for that slice run locally;
the only cross-core communication is a 32KB AllReduce of partial
attention logits within each batch group {0..3}, {4..7}.

v2 changes vs v1 (trace-driven):
  * Q2 free layout -> (dq, eta, kd, n): the q-conv PSUM drain becomes a
    single fully-contiguous 1024-el copy (was 5 ns/el scatter).
  * logits via swapped operands: lhsT = k chunks (32 cols), rhs = Q2
    stream (N=256), 4x column-tiled across PSUM partition strips; strip
    sums + transpose back to [nq, dk] via 4 concurrent fp32 PE
    row-tiles + DVE adds.  Kills 256 128-col LDWEIGHTS.
  * v-transpose done by SBUF->SBUF DMA (128B runs) instead of 128 PE
    transposes + scatter copies; lands directly in the 4-strip layout.
  * attn^T replicated to 4 partition strips with one col-tiled matmul
    quad; attention*V runs 4x row-tiled (contraction dk=32).
  * PSUM evacuation copies alternate between vector and scalar engines.
Output token order is (w', dq, n); host unshard adapts.
"""

import numpy as np
import ml_dtypes

NH, KD, VD, C = 8, 64, 64, 256
B, D, H, W = 2, 32, 32, 32
HS = H // 4            # h rows per core
T = D * HS * W         # 8192 tokens per core
P = 128
NCORES = 8
SCALE = float(VD) ** -0.5

_CACHE = {}


def _build(has_qb, has_kvb, has_pb, sim_mode=False):
    import concourse.bacc as bacc
    import concourse.mybir as mybir
    from concourse import tile

    dt = mybir.dt
    f32, bf16 = dt.float32, dt.bfloat16
    AX = mybir.AxisListType
    AF = mybir.ActivationFunctionType

    nc = bacc.Bacc("TRN2", target_bir_lowering=False, debug=False,
                   enable_asserts=False,
                   num_devices=1 if sim_mode else NCORES)

    x_in = nc.dram_tensor("x", [C, T], bf16, kind="ExternalInput")
    wq_in = nc.dram_tensor("wq", [C, NH * KD], bf16, kind="ExternalInput")
    wkv_in = nc.dram_tensor("wkv", [C, KD + VD], bf16, kind="ExternalInput")
    wp_in = nc.dram_tensor("wp", [NH * VD, C], bf16, kind="ExternalInput")
    idt_in = nc.dram_tensor("idt", [P, P], bf16, kind="ExternalInput")
    idtf_in = nc.dram_tensor("idtf", [P, 32], f32, kind="ExternalInput")
    qb_in = kvb_in = pb_in = None
    if has_qb:
        qb_in = nc.dram_tensor("qb", [P, NH * KD], bf16, kind="ExternalInput")
    if has_kvb:
        kvb_in = nc.dram_tensor("kvb", [P, KD + VD], bf16, kind="ExternalInput")
    if has_pb:
        # proj bias pre-multiplied by layer_scale, per C channel
        pb_in = nc.dram_tensor("pb", [C, 1], f32, kind="ExternalInput")
    out_t = nc.dram_tensor("out", [C, T], f32, kind="ExternalOutput")

    with tile.TileContext(nc) as tc:
        with tc.tile_pool(name="wpool", bufs=1) as wpool, \
             tc.tile_pool(name="big", bufs=1) as bigpool, \
             tc.tile_pool(name="q2p", bufs=1) as q2pool, \
             tc.tile_pool(name="kvp", bufs=1) as kvpool, \
             tc.tile_pool(name="small", bufs=1) as spool, \
             tc.tile_pool(name="stage", bufs=4) as stpool, \
             tc.tile_pool(name="psum", bufs=8, space="PSUM") as psum, \
             tc.tile_pool(name="dram", bufs=1, space="DRAM") as dram:

            # engine rotation for PSUM evacuation copies.
            # (GPSIMD cannot access PSUM, so only vector+scalar rotate.)
            rot_engines = [nc.vector, nc.scalar]
            rot_state = [0]

            def rot_copy(dst, src):
                eng = rot_engines[rot_state[0] % 2]
                rot_state[0] += 1
                if eng is nc.scalar:
                    eng.copy(dst, src)
                else:
                    eng.tensor_copy(dst, src)

            def rot_tt(dst, a, b_, op):
                nc.vector.tensor_tensor(dst, a, b_, op=op)

            # ---- load weights / constants ----
            wq = wpool.tile([P, 2, NH * KD], bf16)
            wkv = wpool.tile([P, 2, KD + VD], bf16)
            wp = wpool.tile([P, 4, C], bf16)
            idt = wpool.tile([P, P], bf16)
            idtf = wpool.tile([P, 32], f32)
            # wkv/wq are needed first; everything else queues behind x so
            # the x chunks (which pace the convs) aren't delayed
            for ci in range(2):
                nc.sync.dma_start(wkv[:, ci, :], wkv_in[ci * P:(ci + 1) * P, :])
                nc.sync.dma_start(wq[:, ci, :], wq_in[ci * P:(ci + 1) * P, :])
            qb = kvb = pb = None
            if has_qb:
                qb = wpool.tile([P, NH * KD], bf16)
                nc.sync.dma_start(qb[:], qb_in[:])
            if has_kvb:
                kvb = wpool.tile([P, KD + VD], bf16)
                nc.sync.dma_start(kvb[:], kvb_in[:])
            if has_pb:
                pb = wpool.tile([P, 2, 1], f32)
                for ci in range(2):
                    nc.sync.dma_start(pb[:, ci, :], pb_in[ci * P:(ci + 1) * P, :])

            # big slot shared sequentially: x (32KB/p) then oo (64KB/p)
            x_sb = bigpool.tile([P, 2, T], bf16, tag="big")
            XCH = 16
            for g in range(XCH):
                lo, hi = g * (T // XCH), (g + 1) * (T // XCH)
                for ci in range(2):
                    eng = nc.sync if ci == 0 else nc.scalar
                    eng.dma_start(x_sb[:, ci, lo:hi],
                                  x_in[ci * P:(ci + 1) * P, lo:hi])
            nc.sync.dma_start(idt[:], idt_in[:])
            nc.sync.dma_start(idtf[:], idtf_in[:])
            for jq in range(4):
                nc.sync.dma_start(wp[:, jq, :], wp_in[jq * P:(jq + 1) * P, :])

            # Q2 [p=hw128, (dq, eta, kd, n)] -- contiguous 1024-el drains
            Q2 = q2pool.tile([P, 32 * 1024], bf16)
            ksb = kvpool.tile([P, 64 * KD], bf16)      # [p=hw128, (dk, eta, kd)]
            vsb = kvpool.tile([P, 64 * VD], bf16)      # [p=hw128, (dk, eta, vd)]
            # vatt4: strip r=[32r..32r+32) holds [dk, (eta, q, b, vd)] for
            # hw128 in [32r, 32r+32);  q = hw128%32 // 2, b = hw128%2
            vatt4 = kvpool.tile([P, 2 * 16 * 2 * VD], bf16)
            attn = spool.tile([P, 2, 32], bf16)
            attnT4 = spool.tile([P, 2, P], bf16)       # attn^T replicated 4 strips
            l2s = spool.tile([P, 256], f32)            # logits strips (dk, nq')
            lsum0 = spool.tile([P, 2, 32], f32)
            lsum1 = spool.tile([P, 2, 32], f32)
            l2 = spool.tile([P, 64], f32)
            l3 = spool.tile([P, 64], f32)
            ex = spool.tile([P, 2, 32], f32)
            red = spool.tile([P, 8], f32)

            arin = [dram.tile([P, 32], f32, name=f"arin{mu}")
                    for mu in range(2)]
            arout = [dram.tile([P, 32], f32, name=f"arout{mu}")
                     for mu in range(2)]

            # ---- kv conv (tokens on partitions), tracking x DMA arrival;
            # q conv for mu0's dq range rides along with its x chunk so
            # logits-mu0/AR0 can trigger right after the last kv chunk ----
            def q_conv(dq):
                for eta in range(2):
                    psq = psum.tile([P, 512], f32, tag="ps",
                                    name=f"psq{dq}_{eta}")
                    j = dq * 2 + eta
                    for ci in range(2):
                        nc.tensor.matmul(psq[:],
                                         x_sb[:, ci, j * P:(j + 1) * P],
                                         wq[:, ci, :],
                                         start=(ci == 0), stop=(ci == 1))
                    # psum free = (kd, n) [wq host col order]; dst is a
                    # contiguous 512-el slice of Q2 (dq, eta, kd, n)
                    dst = Q2[:, dq * 1024 + eta * 512:
                             dq * 1024 + (eta + 1) * 512]
                    if has_qb:
                        rot_tt(dst.rearrange("p c -> p 1 c"),
                               psq.rearrange("p c -> p 1 c"),
                               qb.rearrange("p c -> p 1 c"),
                               mybir.AluOpType.add)
                    else:
                        rot_copy(dst, psq[:])

            for m in range(16):
                ps = psum.tile([P, 512], f32, tag="ps", name=f"pskv{m}")
                for jj in range(4):
                    j = 4 * m + jj
                    for ci in range(2):
                        nc.tensor.matmul(
                            ps[:, jj * P:(jj + 1) * P],
                            x_sb[:, ci, j * P:(j + 1) * P],
                            wkv[:, ci, :],
                            start=(ci == 0), stop=(ci == 1))
                psv = ps.rearrange("p (t c) -> p t c", c=P)
                ks = ksb[:, m * 256:(m + 1) * 256].rearrange("p (t c) -> p t c", c=KD)
                vs = vsb[:, m * 256:(m + 1) * 256].rearrange("p (t c) -> p t c", c=VD)
                if has_kvb:
                    kvbv = kvb.rearrange("p c -> p 1 c")
                    rot_tt(ks, psv[:, :, 0:KD],
                           kvbv[:, [0, 0, 0, 0], 0:KD], mybir.AluOpType.add)
                    rot_tt(vs, psv[:, :, KD:KD + VD],
                           kvbv[:, [0, 0, 0, 0], KD:KD + VD], mybir.AluOpType.add)
                else:
                    rot_copy(ks, psv[:, :, 0:KD])
                    rot_copy(vs, psv[:, :, KD:KD + VD])
                if m < 8:
                    q_conv(2 * m)
                    q_conv(2 * m + 1)

            # ---- v "transpose" into vatt4 via DMA (DRAM bounce so every
            # SBUF AP is partition-first); overlaps the q conv ----
            vtd = [dram.tile([2, 32, 2048], bf16, name=f"vtd{r}")
                   for r in range(4)]
            for r in range(4):
                src1 = vsb[32 * r:32 * (r + 1), :].rearrange(
                    "qb (k e v) -> e qb k v", k=32, e=2, v=VD)
                dst1 = vtd[r].rearrange("e k (qb v) -> e qb k v",
                                        qb=32, v=VD)
                for eta in range(2):
                    eng = nc.sync if (r + eta) % 2 == 0 else nc.scalar
                    eng.dma_start(dst1[eta], src1[eta])
            for r in range(4):
                for eta in range(2):
                    eng = nc.sync if (r + eta) % 2 == 1 else nc.scalar
                    eng.dma_start(
                        vatt4[32 * r:32 * (r + 1),
                              eta * 2048:(eta + 1) * 2048],
                        vtd[r][eta])

            # ---- per-mu: q conv half -> logits half -> AllReduce ----
            # nq' = dq*8+n, so mu = dq-half: logits/AR for mu0 launch after
            # only half the q conv, hiding both AR latencies under compute.
            Q2v = Q2.rearrange("p (dq e k n) -> p e k dq n",
                               dq=32, e=2, k=KD, n=NH)
            ksv = ksb.rearrange("p (dk e k) -> p e k dk", e=2, k=KD)
            for mu in range(2):
                if mu == 1:
                    for dq in range(16, 32):
                        q_conv(dq)

                # logits, 4x col-tiled: lhsT = k chunk (32 cols), rhs = Q2
                # stream over this mu's 16 dq (N=128); strip c_ accumulates
                # (eta,kd) idx in [32c_, 32c_+32).  high_priority: the AR
                # trigger chain must beat the other mu's q conv to the PE.
                hp = tc.high_priority()
                hp.__enter__()
                psL2 = psum.tile([P, P], f32, tag="ps", name=f"psL2_{mu}")
                for step in range(32):
                    for c_ in range(4):
                        idx = c_ * 32 + step
                        eta, kd = idx // KD, idx % KD
                        nc.tensor.matmul(
                            psL2[32 * c_:32 * (c_ + 1), :],
                            ksv[:, eta, kd, :],
                            Q2v[:, eta, kd, mu * 16:(mu + 1) * 16, :],
                            start=(step == 0), stop=(step == 31),
                            tile_position=(0, 32 * c_), skip_group_check=True)
                nc.vector.tensor_copy(l2s[:, mu * P:(mu + 1) * P], psL2[:])

                # strip sums + transpose to [nq', dk] via 4 fp32 row-tiles
                lt = [psum.tile([P, 32], f32, tag="ps", name=f"lt{mu}_{c_}")
                      for c_ in range(4)]
                for c_ in range(4):
                    nc.tensor.matmul(
                        lt[c_][:],
                        l2s[32 * c_:32 * (c_ + 1), mu * P:(mu + 1) * P],
                        idtf[32 * c_:32 * (c_ + 1), :],
                        start=True, stop=True,
                        tile_position=(32 * c_, 0))
                # <=1 PSUM operand per DVE op: stage lt0/lt2 through SBUF
                nc.vector.tensor_copy(lsum0[:, mu, :], lt[0][:])
                nc.scalar.copy(lsum1[:, mu, :], lt[2][:])
                nc.vector.tensor_tensor(lsum0[:, mu, :], lsum0[:, mu, :],
                                        lt[1][:], op=mybir.AluOpType.add)
                nc.vector.tensor_tensor(lsum1[:, mu, :], lsum1[:, mu, :],
                                        lt[3][:], op=mybir.AluOpType.add)
                nc.vector.tensor_tensor(l2[:, mu * 32:(mu + 1) * 32],
                                        lsum0[:, mu, :], lsum1[:, mu, :],
                                        op=mybir.AluOpType.add)
                nc.sync.dma_start(arin[mu][:], l2[:, mu * 32:(mu + 1) * 32])
                if sim_mode:
                    nc.sync.dma_start(arout[mu][:], arin[mu][:])
                else:
                    nc.gpsimd.collective_compute(
                        "AllReduce", mybir.AluOpType.add,
                        replica_groups=[[0, 1, 2, 3], [4, 5, 6, 7]],
                        ins=[arin[mu].opt()], outs=[arout[mu].opt()])
                nc.sync.dma_start(l3[:, mu * 32:(mu + 1) * 32], arout[mu][:])
                hp.__exit__(None, None, None)

            oo = bigpool.tile([P, 4, T], bf16, tag="big", name="oo")
            # oo free per jq plane: f' = w'*256 + nq',  nq' = dq*8 + n
            oov = oo.rearrange("p jq (wh wl n) -> p jq wl wh n", wh=8, wl=4)

            def av_group(mu, eta, qh):
                # tile r holds strip r's outputs for all 4 jq (one PSUM
                # bank per row-tile; concurrent row-tiles never share one)
                pr = [psum.tile([P, 512], f32, tag="ps",
                                name=f"psav{mu}_{eta}_{qh}_{r}")
                      for r in range(4)]
                for jq in range(4):
                    q_ = qh * 4 + jq
                    for r in range(4):
                        nc.tensor.matmul(
                            pr[r][:, jq * P:(jq + 1) * P],
                            vatt4[32 * r:32 * (r + 1),
                                  eta * 2048 + q_ * P:eta * 2048 + (q_ + 1) * P],
                            attnT4[32 * r:32 * (r + 1), mu, :],
                            start=True, stop=True,
                            tile_position=(32 * r, 0))
                for r in range(4):
                    # [p, (jq, nq)] -> oo planes jq at w' = eta*16+4r+qh
                    rot_copy(
                        oov[:, :, qh, eta * 4 + r, mu * P:(mu + 1) * P],
                        pr[r].rearrange("p (jq n) -> p jq n", jq=4))

            # out token order t = mu*4096 + w'*128 + dqloc*8 + n, so the
            # whole mu0 proj + store pipeline runs under the AR1 window
            outv = out_t.rearrange("(ct p) t -> p ct t", p=P)
            oow = [oo[:, jq, :].rearrange("p (w q) -> p w q", w=32)
                   for jq in range(4)]

            def proj_quad(mu, tq):
                # output tokens w' in [4tq, 4tq+4), nq'-half mu
                stg = stpool.tile([P, 2, 512], f32, tag="stg",
                                  name=f"stg{mu}_{tq}")
                for ct in range(2):
                    ps = psum.tile([P, 512], f32, tag="ps",
                                   name=f"psp{mu}_{tq}_{ct}")
                    for jq in range(4):
                        nc.tensor.matmul(
                            ps[:],
                            wp[:, jq, ct * P:(ct + 1) * P],
                            oow[jq][:, 4 * tq:4 * (tq + 1),
                                    mu * P:(mu + 1) * P],
                            start=(jq == 0), stop=(jq == 3))
                    if has_pb:
                        eng = rot_engines[rot_state[0] % 2]
                        rot_state[0] += 1
                        eng.tensor_scalar_add(stg[:, ct, :], ps[:],
                                              pb[:, ct, :])
                    else:
                        rot_copy(stg[:, ct, :], ps[:])
                base = mu * 4096 + tq * 512
                if mu == 1 and tq == 7:
                    for ct in range(2):
                        eng = nc.sync if ct == 0 else nc.scalar
                        eng.dma_start(outv[:, ct, base:base + 512],
                                      stg[:, ct, :])
                else:
                    eng = nc.sync if tq % 2 == 0 else nc.scalar
                    eng.dma_start(outv[:, :, base:base + 512], stg[:])

            for mu in range(2):
                # ---- softmax over dk (free axis) ----
                sl = l3[:, mu * 32:(mu + 1) * 32]
                mx = red[:, mu * 4 + 0: mu * 4 + 1]
                mxn = red[:, mu * 4 + 1: mu * 4 + 2]
                sm = red[:, mu * 4 + 2: mu * 4 + 3]
                rs = red[:, mu * 4 + 3: mu * 4 + 4]
                nc.vector.reduce_max(mx, sl, axis=AX.X, op=mybir.AluOpType.max)
                nc.scalar.mul(mxn, mx, -SCALE)
                nc.scalar.activation(ex[:, mu, :], sl, AF.Exp,
                                     bias=mxn, scale=SCALE, accum_out=sm)
                nc.vector.reciprocal(rs, sm)
                nc.vector.tensor_scalar_mul(attn[:, mu, :], ex[:, mu, :], rs)

                # ---- attn^T replicated to 4 strips (col-tiled quad) ----
                psT = psum.tile([P, P], f32, tag="ps", name=f"psat{mu}")
                for r in range(4):
                    nc.tensor.matmul(psT[32 * r:32 * (r + 1), :],
                                     attn[:, mu, :], idt[:, 0:P],
                                     start=True, stop=True,
                                     tile_position=(0, 32 * r))
                nc.vector.tensor_copy(attnT4[:, mu, :], psT[:])

                # ---- attention * V, then proj per w'-quad ----
                # proj quad tq reads av strip r = tq%4 of eta = tq//4 for
                # all qh, so run all 4 qh groups of an eta, then its projs
                for eta in range(2):
                    for qh in range(4):
                        av_group(mu, eta, qh)
                    for tq in range(eta * 4, (eta + 1) * 4):
                        proj_quad(mu, tq)

    nc.finalize()
    return nc


def _get_nc(has_qb, has_kvb, has_pb, sim_mode=False):
    key = (has_qb, has_kvb, has_pb, sim_mode)
    if key not in _CACHE:
        _CACHE[key] = _build(*key)
    return _CACHE[key]


def _host_inputs(q_w, q_b, kv_w, kv_b, proj_w, proj_b, layer_scale,
                 has_qb, has_kvb, has_pb):
    bf = ml_dtypes.bfloat16
    ls_c = layer_scale.reshape(C)                          # [C] f32
    # wq columns reordered to (kd, n) so the q-conv drain is contiguous
    wq = np.ascontiguousarray(
        q_w.reshape(NH, KD, C).transpose(2, 1, 0).reshape(C, NH * KD)
    ).astype(bf)
    wkv = np.ascontiguousarray(kv_w.T).astype(bf)          # [C, 128]
    wp = np.ascontiguousarray((proj_w * ls_c[:, None]).T).astype(bf)
    idt = np.eye(P, dtype=bf)
    idtf = np.tile(np.eye(32, dtype=np.float32), (4, 1))   # [128, 32]

    shared = {"wq": wq, "wkv": wkv, "wp": wp, "idt": idt, "idtf": idtf}
    if has_qb:
        qbr = q_b.reshape(NH, KD).T.reshape(NH * KD)
        shared["qb"] = np.broadcast_to(qbr.astype(bf), (P, NH * KD)).copy()
    if has_kvb:
        shared["kvb"] = np.broadcast_to(kv_b.astype(bf), (P, KD + VD)).copy()
    if has_pb:
        shared["pb"] = (proj_b * layer_scale.reshape(-1)).reshape(C, 1) \
            .astype(np.float32)
    return shared


def kernel(x, q_w, q_b, kv_w, kv_b, proj_w, proj_b, layer_scale):
    from concourse.bass_utils import run_bass_kernel_spmd
    import os

    x = np.asarray(x, dtype=np.float32)
    q_w = np.asarray(q_w, dtype=np.float32)
    q_b = np.asarray(q_b, dtype=np.float32)
    kv_w = np.asarray(kv_w, dtype=np.float32)
    kv_b = np.asarray(kv_b, dtype=np.float32)
    proj_w = np.asarray(proj_w, dtype=np.float32)
    proj_b = np.asarray(proj_b, dtype=np.float32)
    layer_scale = np.asarray(layer_scale, dtype=np.float32)

    has_qb = bool(np.any(q_b != 0))
    has_kvb = bool(np.any(kv_b != 0))
    has_pb = bool(np.any(proj_b != 0))
    nc = _get_nc(has_qb, has_kvb, has_pb)

    bf = ml_dtypes.bfloat16
    shared = _host_inputs(q_w, q_b, kv_w, kv_b, proj_w, proj_b, layer_scale,
                          has_qb, has_kvb, has_pb)

    in_maps = []
    for c in range(NCORES):
        b, hg = c // 4, c % 4
        xc = np.ascontiguousarray(
            x[b, :, :, hg * HS:(hg + 1) * HS, :].reshape(C, T)).astype(bf)
        in_maps.append({"x": xc, **shared})

    trace = bool(int(os.environ.get("KERNEL_TRACE", "0")))
    res = run_bass_kernel_spmd(nc, in_maps, core_ids=list(range(NCORES)),
                               trace=trace)
    kernel.last_results = res

    out = np.empty((B, C, D, H, W), dtype=np.float32)
    for c in range(NCORES):
        b, hg = c // 4, c % 4
        # out token order: t = mu*4096 + w'*128 + dqloc*8 + n
        r = res.results[c]["out"].reshape(C, 2, W, 16, NH)
        for mu in range(2):
            out[b, :, mu * 16:(mu + 1) * 16, hg::4, :] = \
                r[:, mu].transpose(0, 2, 3, 1)
    return out
